# revision 47
# baseline (speedup 1.0000x reference)
"""Trainium2 Bass kernel for ChamferLossSplitPID.

Contract: kernel(**inputs) takes the FULL inputs (target/reco [64,512,4] f32,
in_pid/out_pid [64,512] i32) and returns the full output (loss_nonzero,
loss_zero) as float32 scalars, matching reference().

Strategy (8 NeuronCores, data-parallel over batch, 8 batches per core):
  The loss only needs distances between SAME-pid pairs, so instead of the
  full [N, N] distance matrix we compute only the 4 diagonal blocks of the
  pid-grouped matrix, in both directions: rows = points of pid p packed
  into a 128-partition chunk (zero-padded lhsT columns -> pad rows produce
  exactly 0, so no row masks are needed anywhere), cols = other-side points
  of pid p padded to a fixed S=128 stride (pad cols produce dist^2 = 2^27,
  never a min winner; the rare group larger than 128 on either side is
  recomputed exactly on the host). dist^2 is a K=16 split-bf16 matmul
  (~1e-5 rel).
  Per (batch, dir) that is 4 matmuls of [16,128]x[16,S] -> one PSUM slot
  each; slots pack 2-per-bank so ONE 4D-AP DVE min-reduce (with
  apply_absolute_value standing in for the relu clamp) drains a whole tile.
  The DVE reduce is the bottleneck engine (PSUM reads are 1 elem/cycle/lane,
  ~1.22us per 8-slot tile); everything else is scheduled around keeping it
  fed without stalls:
    - inputs arrive as 6 DMAs whose sizes/queues are chosen so each batch
      lands just before the DVE consumes it (the HW DGE queue drains ~25+
      GB/s with ~60ns/descriptor, and only ~7 DMA semaphores exist before
      the framework starts serializing on recycled ones);
    - the reduce-tile plan ramps 2/2/4 slots so the first reduce starts
      ~0.3us after batch 0 lands, and ramps down 4/4 so the post-final-
      reduce serial tail is short;
    - per-pid masked norm sums (the only_x/only_y/pid0 branches) live in
      the host epilogue - they are O(B*N) numpy work and shipping norms
      cost 147KB of DMA plus a DVE pass;
    - output is a single [1,64] f32 (256B) DMA issued once at the end.
  Tail: sqrt (Act, 4 pieces overlapping the reduces, bf16 out) -> one
  bf16 ones-matmul partition sum (PE) -> Act copy -> DMA. Rare pid groups
  with >128 members on either side (~0.2% of groups) are recomputed
  exactly on the host and override that (b,pid)'s sums. The tiny O(B*pid)
  epilogue (counts, divisions, empty-group branches, means) runs on the
  host, as does all layout prep (grouping, hi/lo splits, padding).

The emitted IR is input-value-independent (fixed group stride S=128, fixed
128-row chunks), so one SPMD program serves all 8 cores for any input.
"""

import sys

sys.path.insert(0, "/opt/trn_rl_repo")

import numpy as np

from concourse import bacc, bass, mybir, tile
from concourse.bass_utils import run_bass_kernel_spmd

B, N, D = 64, 512, 4
NCORES = 8
BL = B // NCORES          # batches per core
P = 128                   # partitions
NPID = 4                  # nonzero pid classes
BIG = float(2 ** 27)      # pad-column dist^2 (exact in bf16)
KROWS = 16                # split-bf16 contraction rows
NBD = 2 * BL              # (batch, dir) pairs per core
NSLOT = NBD * NPID        # diagonal blocks per core (64)
F32 = mybir.dt.float32
BF16 = mybir.dt.bfloat16

_PROGRAM_CACHE = {}

# Input DMA pieces: (name, queue, [(dir, batch), ...]). The HW DGE queue
# retires ~1 descriptor (row) per ~110ns + ~1.8us pipe latency, so a piece of
# ANY width costs 16 packets and pieces land ~1.8us apart per queue. The two
# queues (0=sync, 1=scalar) are interleaved so the k-th landing on each queue
# arrives just before the DVE consumes those batches (slot order: dir-major,
# batch-major). More/smaller pieces do NOT help: the DMA-sem pool runs out
# at ~8 in-flight DMAs and later DMAs serialize behind recycled semaphores.
# NOTE: row-splitting a piece across both queues was tried and regressed:
# a partition-offset SBUF DMA (tile[8:16,:]) costs ~185ns/descriptor to
# issue vs ~41ns for a full-tile DMA, delaying every later piece.
_PIECES = [
    ("p0", 0, [(0, 0)]),
    ("p1", 1, [(0, 1)]),
    ("p2", 0, [(0, 2)]),
    ("p3", 1, [(0, 3)]),
    ("p4", 1, [(0, 4)]),
    ("p5", 0, [(0, 5), (0, 6), (0, 7)]),
    ("p6", 1, [(1, 0), (1, 1), (1, 2)]),
    ("p7", 1, [(1, b) for b in range(3, 8)]),
]


def _plan_tiles():
    """PSUM tile schedule: (start_slot, nslots) with a small ramp so the
    first DVE reduce starts after only a few matmuls. Slots are 256-element
    (1KB) regions, two per PSUM bank; a full 8-slot tile is 4 banks."""
    # ramp-up aligned to the DMA landings (tile 0 = batch 0, tile 1 =
    # batch 1 — each gated only by its own piece, in their own PSUM pool
    # so the first main tile's matmuls never wait on a reduce), then
    # 7-slot main tiles (512B slot stride, 2 banks x 2 bufs)
    sizes = [4, 4] + [7] * 8
    assert sum(sizes) == NSLOT
    plan = []
    t0 = 0
    for ns in sizes:
        plan.append((t0, ns))
        t0 += ns
    return plan


def _build_program(S: int):
    """Emit the SPMD Bass program for group stride S. Value-independent."""
    COLS = NPID * S
    W = NPID * P + COLS       # lhsT block then rhs block, per batch
    plan = _plan_tiles()
    nc = bacc.Bacc(None)

    # One tensor/DMA per piece. Each consuming Matmult carries a single sync
    # wait (PE LW allows one), so a piece is exactly one DMA.
    d_ab = {nm: nc.dram_tensor(f"ab_{nm}", [KROWS, len(dbs) * W], BF16,
                               kind="ExternalInput")
            for nm, _q, dbs in _PIECES}
    d_sums = nc.dram_tensor("sums", [1, NSLOT], F32, kind="ExternalOutput")

    with tile.TileContext(nc) as tc:
        with (
            tc.tile_pool(name="const", bufs=1) as const,
            tc.tile_pool(name="psum", bufs=2, space=bass.MemorySpace.PSUM) as psum,
            tc.tile_pool(name="ramp", bufs=2, space=bass.MemorySpace.PSUM) as ramp,
        ):
            tAB = {nm: const.tile([KROWS, len(dbs) * W], BF16, tag=f"ab_{nm}",
                                  name=f"tAB_{nm}")
                   for nm, _q, dbs in _PIECES}
            # (dir, batch) -> (piece tile, column base) for operand slicing
            bmap = {}
            for nm, _q, dbs in _PIECES:
                for j, db in enumerate(dbs):
                    bmap[db] = (tAB[nm], j * W)
            # issue in list order; queue 1 (scalar) also hosts the
            # ACT_TABLE_LOADs, which overlap harmlessly.
            for nm, q, _dbs in _PIECES:
                eng = nc.sync if q == 0 else nc.scalar
                eng.dma_start(tAB[nm][:], d_ab[nm][:])
            tONE = const.tile([P, 1], BF16, tag="one")
            nc.vector.memset(tONE[:], 1.0)

            tMS = const.tile([P, NSLOT], F32, tag="ms")   # per-block minima
            # bf16 so the ones-matmul runs in fast bf16 mode instead of the
            # 2-pass fp32 LOW_HIGH mode (sum of ~100 bf16-rounded sqrts:
            # ~1e-4 rel, far inside the 2e-2 gate)
            tSQ = const.tile([P, NSLOT], BF16, tag="sq")  # sqrt'd minima
            tSF = const.tile([1, NSLOT], F32, tag="sf")   # partition sums

            def emit_tile(t0, ns, pool, tag):
                # slot s = dir*32 + batch*4 + group (dir-major). Slots sit at
                # a uniform S(=128)-element 512B stride — 4 per PSUM bank,
                # never straddling one — so ONE 3D-AP DVE reduce drains the
                # whole tile; apply_absolute_value clamps the tiny negative
                # fp-rounding residue (pad rows are exactly 0).
                pt = pool.tile([P, ns, S], F32, tag=tag)
                for i in range(ns):
                    s = t0 + i
                    dr, rem = divmod(s, BL * NPID)
                    b, g = divmod(rem, NPID)
                    tq, base = bmap[(dr, b)]
                    nc.tensor.matmul(
                        pt[:, i, 0:S],
                        tq[:, base + g * P : base + (g + 1) * P],
                        tq[:, base + NPID * P + g * S
                           : base + NPID * P + (g + 1) * S],
                        start=True,
                        stop=True,
                    )
                nc.vector.tensor_reduce(
                    tMS[:, t0 : t0 + ns],
                    pt[:, :, 0:S],
                    axis=mybir.AxisListType.X,
                    op=mybir.AluOpType.min,
                    apply_absolute_value=True,
                )

            def tail_a(lo, hi):
                # pad rows are exactly 0 (zero lhsT cols), so sqrt(0)=0 drops
                # them from the partition sums without any mask.
                nc.scalar.activation(
                    tSQ[:, lo:hi], tMS[:, lo:hi], mybir.ActivationFunctionType.Sqrt)

            def tail_b():
                # one ones-matmul + copy over all 64 columns: the per-piece
                # variant gains nothing because PSUM-pool WAR serializes the
                # po tiles behind the last reduces anyway.
                po = psum.tile([1, NSLOT], F32, tag="dist", name="po")
                nc.tensor.matmul(po[:], tONE[:], tSQ[:, :], start=True, stop=True)
                nc.scalar.copy(tSF[:, :], po[:])

            # tail pieces aligned to reduce-tile edges; the last piece is
            # small so the post-final-reduce serial chain (sqrt -> ones-mm
            # -> copy -> DMA) is as short as possible.
            cuts = [0, 36, 50, NSLOT]
            emitted = 0
            for ti, (t0, ns) in enumerate(plan):
                if ti < 2:
                    emit_tile(t0, ns, ramp, "rdist")
                else:
                    emit_tile(t0, ns, psum, "dist")
                emitted += ns
                for lo, hi in zip(cuts, cuts[1:-1]):
                    if emitted == hi:
                        tail_a(lo, hi)
            tail_a(cuts[-2], NSLOT)
            tail_b()
            # single 256B result DMA once the sums are copied into tSF
            nc.sync.dma_start(d_sums[:, :], tSF[:, :], single_packet=True)

    nc.compile()
    return nc


def _get_program(S: int):
    if S not in _PROGRAM_CACHE:
        _PROGRAM_CACHE[S] = _build_program(S)
    return _PROGRAM_CACHE[S]


def _prep_inputs(target, reco, in_pid, out_pid, S):
    """Build per-core input maps. All heavy compute stays on device; this is
    O(B*N) metadata/layout prep (grouping, norms, hi/lo splits, padding)."""
    COLS = NPID * S
    W = NPID * P + COLS
    t = np.ascontiguousarray(np.asarray(target, dtype=np.float32))
    r = np.ascontiguousarray(np.asarray(reco, dtype=np.float32))
    ip = np.asarray(in_pid)
    op = np.asarray(out_pid)

    import ml_dtypes

    def split16(x):
        hi = x.astype(ml_dtypes.bfloat16).astype(np.float32)
        lo = (x - hi).astype(ml_dtypes.bfloat16).astype(np.float32)
        return hi, lo

    nt2 = (t * t).sum(-1)                      # [B,N]
    nr2 = (r * r).sum(-1)
    ones = np.ones((B, 1, N), np.float32)
    # split-bf16: a.b ~= ahi.bhi + ahi.blo + alo.bhi (lo.lo dropped, ~2^-16 rel)
    # lhsT rows: [(-2x)hi x4, (-2x)hi x4, (-2x)lo x4, |x|2hi, |x|2lo, 1, 1]
    # rhs rows:  [ yhi x4,     ylo x4,     yhi x4,    1,      1, |y|2hi, |y|2lo]
    def build_lhs(x, x2):
        m2hi, m2lo = split16(-2.0 * x.transpose(0, 2, 1))   # [B,4,N]
        x2hi, x2lo = split16(x2[:, None, :])                # [B,1,N]
        return np.concatenate(
            [m2hi, m2hi, m2lo, x2hi, x2lo, ones, ones], axis=1)  # [B,16,N]

    Lt = build_lhs(t, nt2)
    Lr = build_lhs(r, nr2)
    thi, tlo = split16(t)
    rhi, rlo = split16(r)
    t2hi, t2lo = split16(nt2)
    r2hi, r2lo = split16(nr2)

    AB = np.zeros((2, B, KROWS, W), np.float32)
    sides = [(Lt, ip, rhi, rlo, r2hi, r2lo, op),   # dir0: rows targets, cols recos
             (Lr, op, thi, tlo, t2hi, t2lo, ip)]   # dir1: rows recos, cols targets
    for dirn, (xL, xpid, yhi, ylo, y2hi, y2lo, ypid) in enumerate(sides):
        for b in range(B):
            for g in range(NPID):
                p = g + 1
                ridx = np.nonzero(xpid[b] == p)[0][:P]
                AB[dirn, b, :, g * P : g * P + len(ridx)] = xL[b][:, ridx]
                cidx = np.nonzero(ypid[b] == p)[0][:S]
                c0 = NPID * P + g * S
                k = len(cidx)
                AB[dirn, b, 0:4, c0 : c0 + k] = yhi[b, cidx].T
                AB[dirn, b, 4:8, c0 : c0 + k] = ylo[b, cidx].T
                AB[dirn, b, 8:12, c0 : c0 + k] = yhi[b, cidx].T
                AB[dirn, b, 12:14, c0 : c0 + k] = 1.0
                AB[dirn, b, 14, c0 : c0 + k] = y2hi[b, cidx]
                AB[dirn, b, 15, c0 : c0 + k] = y2lo[b, cidx]
                AB[dirn, b, 14, c0 + k : c0 + S] = BIG

    in_maps = []
    for ci in range(NCORES):
        m = {}
        for nm, _q, dbs in _PIECES:
            blk = [AB[d, ci * BL + b] for d, b in dbs]
            m[f"ab_{nm}"] = np.ascontiguousarray(
                np.concatenate(blk, axis=1).astype(ml_dtypes.bfloat16))
        in_maps.append(m)
    return in_maps


def _overflow_overrides(t, r, ip, op, S):
    """Exact fp32 host recompute for pid groups that exceed the device's
    fixed capacity (128 row-partitions / S columns): the device block only
    covers the first 128/S members, so both direction sums for that (b,p)
    are replaced wholesale. Binomial(512, 0.2) exceeds 128 for ~0.2% of
    groups, so this is ~one small [k,k] block per input."""
    overrides = {}
    for b in range(B):
        for p in range(1, 5):
            ridx = np.nonzero(ip[b] == p)[0]
            cidx = np.nonzero(op[b] == p)[0]
            if (len(ridx) > P or len(cidx) > S) and len(ridx) and len(cidx):
                d2 = ((t[b, ridx][:, None, :] - r[b, cidx][None, :, :]) ** 2
                      ).sum(-1)
                d = np.sqrt(d2)
                overrides[(b, p)] = (d.min(1).sum(), d.min(0).sum())
    return overrides


def _epilogue(sums_all, t, r, ip, op, S):
    """Tiny O(B*pid) final combination, mirrors reference()'s branch logic.
    The masked norm sums (only_x/only_y/pid0 zero-loss) are O(B*N) numpy —
    cheaper recomputed here than shipped to and reduced on the device."""
    sum_xy = np.zeros((B, 5))
    sum_yx = np.zeros((B, 5))
    for ci in range(NCORES):
        srow = sums_all[ci].reshape(2, BL, NPID)   # slot = dir*32 + b*4 + g
        for lb in range(BL):
            b = ci * BL + lb
            sum_xy[b, 1:5] = srow[0, lb]
            sum_yx[b, 1:5] = srow[1, lb]

    for (b, p), (sxy, syx) in _overflow_overrides(t, r, ip, op, S).items():
        sum_xy[b, p] = sxy
        sum_yx[b, p] = syx

    normt = np.sqrt((t * t).sum(-1)).astype(np.float32)
    normr = np.sqrt((r * r).sum(-1)).astype(np.float32)
    cx = np.stack([(ip == p).sum(1) for p in range(5)], 1)  # [B,5]
    cy = np.stack([(op == p).sum(1) for p in range(5)], 1)

    loss_nonzero = np.float32(0.0)
    for p in range(1, 5):
        both = 0.5 * (sum_xy[:, p] / np.maximum(1, cy[:, p])
                      + sum_yx[:, p] / np.maximum(1, cx[:, p]))
        ox = (normt * (ip == p)).sum(1) / np.maximum(1, cx[:, p])  # y empty
        oy = (normr * (op == p)).sum(1) / np.maximum(1, cy[:, p])  # x empty
        per_b = np.where(cy[:, p] == 0, ox, np.where(cx[:, p] == 0, oy, both))
        loss_nonzero = loss_nonzero + np.float32(per_b.mean())
    mask0 = (op == 0)
    loss_zero = np.float32(
        ((normr * mask0).sum(1) / np.maximum(1, mask0.sum(1))).mean())
    return np.float32(loss_nonzero), np.float32(loss_zero)


def kernel(target, reco, in_pid, out_pid):
    t = np.ascontiguousarray(np.asarray(target, dtype=np.float32))
    r = np.ascontiguousarray(np.asarray(reco, dtype=np.float32))
    ip = np.asarray(in_pid)
    op = np.asarray(out_pid)
    # Fixed group stride: groups larger than S (or 128 rows) are recomputed
    # exactly on the host (_overflow_overrides), so the program never needs
    # a value-dependent recompile.
    S = 128

    nc = _get_program(S)
    in_maps = _prep_inputs(t, r, ip, op, S)
    res = run_bass_kernel_spmd(nc, in_maps, list(range(NCORES)))
    sums_all = [res.results[ci]["sums"] for ci in range(NCORES)]
    return _epilogue(sums_all, t, r, ip, op, S)


# revision 48
# speedup vs baseline: 1.0191x; 1.0191x over previous
"""Trainium2 Bass kernel for ChamferLossSplitPID.

Contract: kernel(**inputs) takes the FULL inputs (target/reco [64,512,4] f32,
in_pid/out_pid [64,512] i32) and returns the full output (loss_nonzero,
loss_zero) as float32 scalars, matching reference().

Strategy (8 NeuronCores, data-parallel over batch, 8 batches per core):
  The loss only needs distances between SAME-pid pairs, so instead of the
  full [N, N] distance matrix we compute only the 4 diagonal blocks of the
  pid-grouped matrix, in both directions: rows = points of pid p packed
  into a 128-partition chunk (zero-padded lhsT columns -> pad rows produce
  exactly 0, so no row masks are needed anywhere), cols = other-side points
  of pid p padded to a fixed S=128 stride (pad cols produce dist^2 = 2^27,
  never a min winner; the rare group larger than 128 on either side is
  recomputed exactly on the host). dist^2 is a K=16 split-bf16 matmul
  (~1e-5 rel).
  Per (batch, dir) that is 4 matmuls of [16,128]x[16,S] -> one PSUM slot
  each; slots pack 2-per-bank so ONE 4D-AP DVE min-reduce (with
  apply_absolute_value standing in for the relu clamp) drains a whole tile.
  The DVE reduce is the bottleneck engine (PSUM reads are 1 elem/cycle/lane,
  ~1.22us per 8-slot tile); everything else is scheduled around keeping it
  fed without stalls:
    - inputs arrive as 6 DMAs whose sizes/queues are chosen so each batch
      lands just before the DVE consumes it (the HW DGE queue drains ~25+
      GB/s with ~60ns/descriptor, and only ~7 DMA semaphores exist before
      the framework starts serializing on recycled ones);
    - the reduce-tile plan ramps 2/2/4 slots so the first reduce starts
      ~0.3us after batch 0 lands, and ramps down 4/4 so the post-final-
      reduce serial tail is short;
    - per-pid masked norm sums (the only_x/only_y/pid0 branches) live in
      the host epilogue - they are O(B*N) numpy work and shipping norms
      cost 147KB of DMA plus a DVE pass;
    - output is a single [1,64] f32 (256B) DMA issued once at the end.
  Tail: sqrt (Act, 4 pieces overlapping the reduces, bf16 out) -> one
  bf16 ones-matmul partition sum (PE) -> Act copy -> DMA. Rare pid groups
  with >128 members on either side (~0.2% of groups) are recomputed
  exactly on the host and override that (b,pid)'s sums. The tiny O(B*pid)
  epilogue (counts, divisions, empty-group branches, means) runs on the
  host, as does all layout prep (grouping, hi/lo splits, padding).

The emitted IR is input-value-independent (fixed group stride S=128, fixed
128-row chunks), so one SPMD program serves all 8 cores for any input.
"""

import sys

sys.path.insert(0, "/opt/trn_rl_repo")

import numpy as np

from concourse import bacc, bass, mybir, tile
from concourse.bass_utils import run_bass_kernel_spmd

B, N, D = 64, 512, 4
NCORES = 8
BL = B // NCORES          # batches per core
P = 128                   # partitions
NPID = 4                  # nonzero pid classes
BIG = float(2 ** 27)      # pad-column dist^2 (exact in bf16)
KROWS = 16                # split-bf16 contraction rows
NBD = 2 * BL              # (batch, dir) pairs per core
NSLOT = NBD * NPID        # diagonal blocks per core (64)
F32 = mybir.dt.float32
BF16 = mybir.dt.bfloat16

_PROGRAM_CACHE = {}

# Input DMA pieces: (name, queue, [(dir, batch), ...]). The HW DGE queue
# retires ~1 descriptor (row) per ~110ns + ~1.8us pipe latency, so a piece of
# ANY width costs 16 packets and pieces land ~1.8us apart per queue. The two
# queues (0=sync, 1=scalar) are interleaved so the k-th landing on each queue
# arrives just before the DVE consumes those batches (slot order: dir-major,
# batch-major). More/smaller pieces do NOT help: the DMA-sem pool runs out
# at ~8 in-flight DMAs and later DMAs serialize behind recycled semaphores.
# NOTE: row-splitting a piece across both queues was tried and regressed:
# a partition-offset SBUF DMA (tile[8:16,:]) costs ~185ns/descriptor to
# issue vs ~41ns for a full-tile DMA, delaying every later piece.
# 8-piece variants (splitting b2/b3 to chase the last ~0.4us r1->r2 gap)
# were tried and regressed: the extra DMA instruction issue time on the
# scalar queue pushes later landings back more than the early gap saves.
_PIECES = [
    ("p0", 0, [(0, 0)]),
    ("p1", 1, [(0, 1)]),
    ("p2", 0, [(0, 2), (0, 3)]),
    ("p3", 1, [(0, 4)]),
    ("p4", 0, [(0, 5), (0, 6), (0, 7)]),
    ("p5", 1, [(1, 0), (1, 1), (1, 2)]),
    ("p6", 1, [(1, b) for b in range(3, 8)]),
]


def _plan_tiles():
    """PSUM tile schedule: (start_slot, nslots) with a small ramp so the
    first DVE reduce starts after only a few matmuls. Slots are 256-element
    (1KB) regions, two per PSUM bank; a full 8-slot tile is 4 banks."""
    # ramp-up aligned to the DMA landings (tile 0 = batch 0, tile 1 =
    # batch 1 — each gated only by its own piece, in their own PSUM pool
    # so the first main tile's matmuls never wait on a reduce), then
    # 7-slot main tiles (512B slot stride, 2 banks x 2 bufs)
    sizes = [4, 4] + [7] * 8
    assert sum(sizes) == NSLOT
    plan = []
    t0 = 0
    for ns in sizes:
        plan.append((t0, ns))
        t0 += ns
    return plan


def _build_program(S: int):
    """Emit the SPMD Bass program for group stride S. Value-independent."""
    COLS = NPID * S
    W = NPID * P + COLS       # lhsT block then rhs block, per batch
    plan = _plan_tiles()
    nc = bacc.Bacc(None)

    # One tensor/DMA per piece. Each consuming Matmult carries a single sync
    # wait (PE LW allows one), so a piece is exactly one DMA.
    d_ab = {nm: nc.dram_tensor(f"ab_{nm}", [KROWS, len(dbs) * W], BF16,
                               kind="ExternalInput")
            for nm, _q, dbs in _PIECES}
    d_sums = nc.dram_tensor("sums", [1, NSLOT], F32, kind="ExternalOutput")

    with tile.TileContext(nc) as tc:
        with (
            tc.tile_pool(name="const", bufs=1) as const,
            tc.tile_pool(name="psum", bufs=2, space=bass.MemorySpace.PSUM) as psum,
            tc.tile_pool(name="ramp", bufs=2, space=bass.MemorySpace.PSUM) as ramp,
        ):
            tAB = {nm: const.tile([KROWS, len(dbs) * W], BF16, tag=f"ab_{nm}",
                                  name=f"tAB_{nm}")
                   for nm, _q, dbs in _PIECES}
            # (dir, batch) -> (piece tile, column base) for operand slicing
            bmap = {}
            for nm, _q, dbs in _PIECES:
                for j, db in enumerate(dbs):
                    bmap[db] = (tAB[nm], j * W)
            # issue in list order; queue 1 (scalar) also hosts the
            # ACT_TABLE_LOADs, which overlap harmlessly.
            for nm, q, _dbs in _PIECES:
                eng = nc.sync if q == 0 else nc.scalar
                eng.dma_start(tAB[nm][:], d_ab[nm][:])
            tONE = const.tile([P, 1], BF16, tag="one")
            nc.vector.memset(tONE[:], 1.0)

            tMS = const.tile([P, NSLOT], F32, tag="ms")   # per-block minima
            # bf16 so the ones-matmul runs in fast bf16 mode instead of the
            # 2-pass fp32 LOW_HIGH mode (sum of ~100 bf16-rounded sqrts:
            # ~1e-4 rel, far inside the 2e-2 gate)
            tSQ = const.tile([P, NSLOT], BF16, tag="sq")  # sqrt'd minima
            tSF = const.tile([1, NSLOT], F32, tag="sf")   # partition sums

            def emit_tile(t0, ns, pool, tag):
                # slot s = dir*32 + batch*4 + group (dir-major). Slots sit at
                # a uniform S(=128)-element 512B stride — 4 per PSUM bank,
                # never straddling one — so ONE 3D-AP DVE reduce drains the
                # whole tile; apply_absolute_value clamps the tiny negative
                # fp-rounding residue (pad rows are exactly 0).
                pt = pool.tile([P, ns, S], F32, tag=tag)
                for i in range(ns):
                    s = t0 + i
                    dr, rem = divmod(s, BL * NPID)
                    b, g = divmod(rem, NPID)
                    tq, base = bmap[(dr, b)]
                    nc.tensor.matmul(
                        pt[:, i, 0:S],
                        tq[:, base + g * P : base + (g + 1) * P],
                        tq[:, base + NPID * P + g * S
                           : base + NPID * P + (g + 1) * S],
                        start=True,
                        stop=True,
                    )
                nc.vector.tensor_reduce(
                    tMS[:, t0 : t0 + ns],
                    pt[:, :, 0:S],
                    axis=mybir.AxisListType.X,
                    op=mybir.AluOpType.min,
                    apply_absolute_value=True,
                )

            def tail_a(lo, hi):
                # pad rows are exactly 0 (zero lhsT cols), so sqrt(0)=0 drops
                # them from the partition sums without any mask.
                nc.scalar.activation(
                    tSQ[:, lo:hi], tMS[:, lo:hi], mybir.ActivationFunctionType.Sqrt)

            def tail_b():
                # one ones-matmul + copy over all 64 columns: the per-piece
                # variant gains nothing because PSUM-pool WAR serializes the
                # po tiles behind the last reduces anyway.
                po = psum.tile([1, NSLOT], F32, tag="dist", name="po")
                nc.tensor.matmul(po[:], tONE[:], tSQ[:, :], start=True, stop=True)
                nc.scalar.copy(tSF[:, :], po[:])

            # tail pieces aligned to reduce-tile edges; the last piece is
            # small so the post-final-reduce serial chain (sqrt -> ones-mm
            # -> copy -> DMA) is as short as possible.
            cuts = [0, 36, 50, NSLOT]
            emitted = 0
            for ti, (t0, ns) in enumerate(plan):
                if ti < 2:
                    emit_tile(t0, ns, ramp, "rdist")
                else:
                    emit_tile(t0, ns, psum, "dist")
                emitted += ns
                for lo, hi in zip(cuts, cuts[1:-1]):
                    if emitted == hi:
                        tail_a(lo, hi)
            tail_a(cuts[-2], NSLOT)
            tail_b()
            # single 256B result DMA once the sums are copied into tSF
            nc.sync.dma_start(d_sums[:, :], tSF[:, :], single_packet=True)

    nc.compile()
    return nc


def _get_program(S: int):
    if S not in _PROGRAM_CACHE:
        _PROGRAM_CACHE[S] = _build_program(S)
    return _PROGRAM_CACHE[S]


def _prep_inputs(target, reco, in_pid, out_pid, S):
    """Build per-core input maps. All heavy compute stays on device; this is
    O(B*N) metadata/layout prep (grouping, norms, hi/lo splits, padding)."""
    COLS = NPID * S
    W = NPID * P + COLS
    t = np.ascontiguousarray(np.asarray(target, dtype=np.float32))
    r = np.ascontiguousarray(np.asarray(reco, dtype=np.float32))
    ip = np.asarray(in_pid)
    op = np.asarray(out_pid)

    import ml_dtypes

    def split16(x):
        hi = x.astype(ml_dtypes.bfloat16).astype(np.float32)
        lo = (x - hi).astype(ml_dtypes.bfloat16).astype(np.float32)
        return hi, lo

    nt2 = (t * t).sum(-1)                      # [B,N]
    nr2 = (r * r).sum(-1)
    ones = np.ones((B, 1, N), np.float32)
    # split-bf16: a.b ~= ahi.bhi + ahi.blo + alo.bhi (lo.lo dropped, ~2^-16 rel)
    # lhsT rows: [(-2x)hi x4, (-2x)hi x4, (-2x)lo x4, |x|2hi, |x|2lo, 1, 1]
    # rhs rows:  [ yhi x4,     ylo x4,     yhi x4,    1,      1, |y|2hi, |y|2lo]
    def build_lhs(x, x2):
        m2hi, m2lo = split16(-2.0 * x.transpose(0, 2, 1))   # [B,4,N]
        x2hi, x2lo = split16(x2[:, None, :])                # [B,1,N]
        return np.concatenate(
            [m2hi, m2hi, m2lo, x2hi, x2lo, ones, ones], axis=1)  # [B,16,N]

    Lt = build_lhs(t, nt2)
    Lr = build_lhs(r, nr2)
    thi, tlo = split16(t)
    rhi, rlo = split16(r)
    t2hi, t2lo = split16(nt2)
    r2hi, r2lo = split16(nr2)

    AB = np.zeros((2, B, KROWS, W), np.float32)
    sides = [(Lt, ip, rhi, rlo, r2hi, r2lo, op),   # dir0: rows targets, cols recos
             (Lr, op, thi, tlo, t2hi, t2lo, ip)]   # dir1: rows recos, cols targets
    for dirn, (xL, xpid, yhi, ylo, y2hi, y2lo, ypid) in enumerate(sides):
        for b in range(B):
            for g in range(NPID):
                p = g + 1
                ridx = np.nonzero(xpid[b] == p)[0][:P]
                AB[dirn, b, :, g * P : g * P + len(ridx)] = xL[b][:, ridx]
                cidx = np.nonzero(ypid[b] == p)[0][:S]
                c0 = NPID * P + g * S
                k = len(cidx)
                AB[dirn, b, 0:4, c0 : c0 + k] = yhi[b, cidx].T
                AB[dirn, b, 4:8, c0 : c0 + k] = ylo[b, cidx].T
                AB[dirn, b, 8:12, c0 : c0 + k] = yhi[b, cidx].T
                AB[dirn, b, 12:14, c0 : c0 + k] = 1.0
                AB[dirn, b, 14, c0 : c0 + k] = y2hi[b, cidx]
                AB[dirn, b, 15, c0 : c0 + k] = y2lo[b, cidx]
                AB[dirn, b, 14, c0 + k : c0 + S] = BIG

    in_maps = []
    for ci in range(NCORES):
        m = {}
        for nm, _q, dbs in _PIECES:
            blk = [AB[d, ci * BL + b] for d, b in dbs]
            m[f"ab_{nm}"] = np.ascontiguousarray(
                np.concatenate(blk, axis=1).astype(ml_dtypes.bfloat16))
        in_maps.append(m)
    return in_maps


def _overflow_overrides(t, r, ip, op, S):
    """Exact fp32 host recompute for pid groups that exceed the device's
    fixed capacity (128 row-partitions / S columns): the device block only
    covers the first 128/S members, so both direction sums for that (b,p)
    are replaced wholesale. Binomial(512, 0.2) exceeds 128 for ~0.2% of
    groups, so this is ~one small [k,k] block per input."""
    overrides = {}
    for b in range(B):
        for p in range(1, 5):
            ridx = np.nonzero(ip[b] == p)[0]
            cidx = np.nonzero(op[b] == p)[0]
            if (len(ridx) > P or len(cidx) > S) and len(ridx) and len(cidx):
                d2 = ((t[b, ridx][:, None, :] - r[b, cidx][None, :, :]) ** 2
                      ).sum(-1)
                d = np.sqrt(d2)
                overrides[(b, p)] = (d.min(1).sum(), d.min(0).sum())
    return overrides


def _epilogue(sums_all, t, r, ip, op, S):
    """Tiny O(B*pid) final combination, mirrors reference()'s branch logic.
    The masked norm sums (only_x/only_y/pid0 zero-loss) are O(B*N) numpy —
    cheaper recomputed here than shipped to and reduced on the device."""
    sum_xy = np.zeros((B, 5))
    sum_yx = np.zeros((B, 5))
    for ci in range(NCORES):
        srow = sums_all[ci].reshape(2, BL, NPID)   # slot = dir*32 + b*4 + g
        for lb in range(BL):
            b = ci * BL + lb
            sum_xy[b, 1:5] = srow[0, lb]
            sum_yx[b, 1:5] = srow[1, lb]

    for (b, p), (sxy, syx) in _overflow_overrides(t, r, ip, op, S).items():
        sum_xy[b, p] = sxy
        sum_yx[b, p] = syx

    normt = np.sqrt((t * t).sum(-1)).astype(np.float32)
    normr = np.sqrt((r * r).sum(-1)).astype(np.float32)
    cx = np.stack([(ip == p).sum(1) for p in range(5)], 1)  # [B,5]
    cy = np.stack([(op == p).sum(1) for p in range(5)], 1)

    loss_nonzero = np.float32(0.0)
    for p in range(1, 5):
        both = 0.5 * (sum_xy[:, p] / np.maximum(1, cy[:, p])
                      + sum_yx[:, p] / np.maximum(1, cx[:, p]))
        ox = (normt * (ip == p)).sum(1) / np.maximum(1, cx[:, p])  # y empty
        oy = (normr * (op == p)).sum(1) / np.maximum(1, cy[:, p])  # x empty
        per_b = np.where(cy[:, p] == 0, ox, np.where(cx[:, p] == 0, oy, both))
        loss_nonzero = loss_nonzero + np.float32(per_b.mean())
    mask0 = (op == 0)
    loss_zero = np.float32(
        ((normr * mask0).sum(1) / np.maximum(1, mask0.sum(1))).mean())
    return np.float32(loss_nonzero), np.float32(loss_zero)


def kernel(target, reco, in_pid, out_pid):
    t = np.ascontiguousarray(np.asarray(target, dtype=np.float32))
    r = np.ascontiguousarray(np.asarray(reco, dtype=np.float32))
    ip = np.asarray(in_pid)
    op = np.asarray(out_pid)
    # Fixed group stride: groups larger than S (or 128 rows) are recomputed
    # exactly on the host (_overflow_overrides), so the program never needs
    # a value-dependent recompile.
    S = 128

    nc = _get_program(S)
    in_maps = _prep_inputs(t, r, ip, op, S)
    res = run_bass_kernel_spmd(nc, in_maps, list(range(NCORES)))
    sums_all = [res.results[ci]["sums"] for ci in range(NCORES)]
    return _epilogue(sums_all, t, r, ip, op, S)
